# revision 7
# baseline (speedup 1.0000x reference)
"""Paged-attention decode (GQA) on 8 Trainium2 NeuronCores.

Strategy (data-parallel over 128-token tiles):
  - Host gathers each sequence's valid KV blocks (via block_table/seq_lens)
    into packed 128-token tiles: K transposed to [D=128, L] per KV head,
    V natural [L, D=128] per KV head, plus a validity column (for the
    softmax denominator matmul).
  - Tiles are distributed evenly across the 8 cores (each tile = same cost).
  - Precision: the kernel is HBM-bandwidth bound, so KV bytes are
    everything. Sequences with L >= 512 tokens ship K/V in fp8 (e3m4:
    4 mantissa bits); shorter sequences (whose softmax averages over
    fewer tokens and so amplifies quantization noise the most) stay in
    bf16. q and p (the exp'd scores) stay bf16 -- the tensor engine
    accepts mixed-dtype operands. Accumulation is fp32 PSUM; the final
    combine runs on host in float64. End-to-end rel err ~1.1e-2
    (gate 2e-2) -- validated offline against the fp64 reference; the
    bf16-only variant of this pipeline reproduced its offline sim
    error to 4 digits on hardware.
  - No masking is needed: padded tokens have K=V=0 so scores=0, p=1,
    but V=0 keeps them out of the numerator and the valid column keeps
    them out of the denominator.
  - Device, per tile: 8 QK matmuls (K_h stationary, q streams) ->
    scores [128L, 32hg] in PSUM, one ScalarE exp -> p bf16, 8 PV
    matmuls (V_h stationary, p streams) + 1 denominator matmul into
    acc [128, 33] PSUM, DVE copy to an SBUF staging buffer. KV streams
    in ~1-2 MiB DMA chunks; finished outputs stream back incrementally.
  - Host sums per-tile partial numerators/denominators per sequence and
    normalizes (the standard distributed-softmax combine).
"""

import math

import numpy as np

# Problem constants (hardcoded per task contract).
NUM_SEQS = 32
NUM_HEADS = 32
NUM_KV_HEADS = 8
GQA = NUM_HEADS // NUM_KV_HEADS  # 4
HEAD_SIZE = 128
BLOCK_SIZE = 16
MAX_BLOCKS_PER_SEQ = 128
MAX_SEQ_LEN = MAX_BLOCKS_PER_SEQ * BLOCK_SIZE
SCALE = 1.0 / math.sqrt(HEAD_SIZE)
N_CORES = 8
TILE_L = 128          # tokens per device tile
FP8_MIN_L = 512       # sequences at least this long ship KV in fp8
HG = NUM_HEADS        # 32 (kv_head-major query head order)
HB = NUM_KV_HEADS * HEAD_SIZE      # 1024 cols per K/V plane
KV_COLS = 2 * HB + 2               # 2050: K | V | valid | pad

_PROGRAM_CACHE = {}
LAST_RUN = None  # BassKernelResults of the most recent run (for test harness)


def _build_program(n16: int, n8: int):
    """Build the SPMD Bass/Tile program: per core, n16 bf16 KV tiles
    followed by n8 fp8(e3m4) KV tiles."""
    import concourse.bacc as bacc
    import concourse.mybir as mybir
    import concourse.tile as tile

    f32 = mybir.dt.float32
    bf16 = mybir.dt.bfloat16
    fp8 = mybir.dt.float8e3
    nt = n16 + n8
    nc = bacc.Bacc("TRN2", target_bir_lowering=False, debug=False,
                   num_devices=N_CORES)

    kv16_d = nc.dram_tensor("kv16", [128, max(n16, 1) * KV_COLS], bf16,
                            kind="ExternalInput")
    kv8_d = nc.dram_tensor("kv8", [128, max(n8, 1) * KV_COLS], fp8,
                           kind="ExternalInput")
    q_d = nc.dram_tensor("q", [128, nt * HG], bf16, kind="ExternalInput")
    out_d = nc.dram_tensor("out", [128, nt * (HG + 1)], f32,
                           kind="ExternalOutput")

    with tile.TileContext(nc) as tc:
        with (
            tc.tile_pool(name="const", bufs=1) as const_pool,
            tc.tile_pool(name="kv16p", bufs=2) as kv16_pool,
            tc.tile_pool(name="kv8p", bufs=4) as kv8_pool,
            tc.tile_pool(name="pp", bufs=4) as p_pool,
            tc.tile_pool(name="acc_sb", bufs=1) as stage_pool,
            tc.tile_pool(name="ps_s", bufs=4, space="PSUM") as ps_scores,
            tc.tile_pool(name="ps_o", bufs=4, space="PSUM") as ps_acc,
        ):
            # q first on the same (sync/HWDGE) queue as KV so it does not
            # compete with the KV stream for HBM bandwidth mid-kernel.
            qt = const_pool.tile([128, nt * HG], bf16)
            nc.sync.dma_start(out=qt[:], in_=q_d.ap())
            out_stage = stage_pool.tile([128, nt * (HG + 1)], f32)
            nc.vector.memset(out_stage[:], 0.0)

            # DMA chunk schedule. bf16 tiles (if any) come first in one
            # chunk; fp8 tiles stream in 8-tile (~2 MiB) chunks for DMA
            # efficiency, tapering to 4/2/1-tile chunks at the end to
            # shorten the pipeline drain.
            sizes = []
            r = n8
            while r > 11:
                sizes.append(8)
                r -= 8
            sizes += {11: [8, 2, 1], 10: [4, 4, 1, 1], 9: [4, 2, 2, 1],
                      8: [4, 2, 1, 1], 7: [4, 2, 1], 6: [2, 2, 1, 1],
                      5: [2, 2, 1], 4: [2, 1, 1], 3: [2, 1],
                      2: [1, 1], 1: [1], 0: []}[r]
            starts = [sum(sizes[:i]) for i in range(len(sizes))]

            chunk_tiles = {}
            if n16:
                ct = kv16_pool.tile([128, n16 * KV_COLS], bf16)
                nc.sync.dma_start(out=ct[:], in_=kv16_d.ap())
                for i in range(n16):
                    chunk_tiles[i] = ct[:, i * KV_COLS:(i + 1) * KV_COLS]
            big = max(sizes) if sizes else 1
            for ci, (sz, st) in enumerate(zip(sizes, starts)):
                ct = kv8_pool.tile([128, big * KV_COLS], fp8)
                c0 = st * KV_COLS
                if ci >= len(sizes) - 2 and sz == 1:
                    # split the last tiles' DMA into K-plane then V-plane
                    # so their QK matmuls overlap the V transfer
                    # (shortens the end-of-kernel serial drain)
                    nc.sync.dma_start(
                        out=ct[:, :HB],
                        in_=kv8_d.ap()[:, c0:c0 + HB])
                    nc.sync.dma_start(
                        out=ct[:, HB:KV_COLS],
                        in_=kv8_d.ap()[:, c0 + HB:c0 + KV_COLS])
                else:
                    nc.sync.dma_start(
                        out=ct[:, :sz * KV_COLS],
                        in_=kv8_d.ap()[:, c0:c0 + sz * KV_COLS])
                for i in range(sz):
                    chunk_tiles[n16 + st + i] = ct[:, i * KV_COLS:
                                                   (i + 1) * KV_COLS]

            OUT_CHUNK = 8  # tiles per incremental output store
            out_done = 0   # tiles whose output has been stored

            for t in range(nt):
                kvt = chunk_tiles[t]

                # scores[l, h*4+g] = sum_d K[l,d] * q_scaled[h,g,d]
                scores = ps_scores.tile([128, HG], f32)
                qb = t * HG
                for h in range(NUM_KV_HEADS):
                    nc.tensor.matmul(
                        scores[:, h * GQA:(h + 1) * GQA],
                        kvt[:, h * HEAD_SIZE:(h + 1) * HEAD_SIZE],
                        qt[:, qb + h * GQA:qb + (h + 1) * GQA],
                        start=True, stop=True)

                # p = exp(scores), emitted directly in bf16
                p = p_pool.tile([128, HG], bf16)
                nc.scalar.activation(
                    p[:], scores[:], mybir.ActivationFunctionType.Exp)

                # acc[d, h*4+g] = sum_l V[l, h, d] * p[l, h*4+g]
                # acc[0:32, 32] = per-(h,g) denominator sum_l p[l,:]*valid[l]
                acc = ps_acc.tile([128, HG + 1], f32)
                for h in range(NUM_KV_HEADS):
                    nc.tensor.matmul(
                        acc[:, h * GQA:(h + 1) * GQA],
                        kvt[:, HB + h * HEAD_SIZE:HB + (h + 1) * HEAD_SIZE],
                        p[:, h * GQA:(h + 1) * GQA],
                        start=True, stop=True)
                valid = kvt[:, KV_COLS - 2:KV_COLS - 1]
                nc.tensor.matmul(acc[0:HG, HG:HG + 1], p[:], valid,
                                 start=True, stop=True)

                base = t * (HG + 1)
                # single copy; rows 32-127 of the denominator column are
                # PSUM garbage, skipped by the host combine
                nc.vector.tensor_copy(
                    out_stage[:, base:base + HG + 1], acc[:])

                # stream finished output chunks while KV is still loading;
                # per-tile stores for the last tiles so the final DMA only
                # waits on the last tile's copy and moves a few KB
                emit = (t % OUT_CHUNK == OUT_CHUNK - 1 or t >= nt - 3)
                if emit:
                    c0 = out_done * (HG + 1)
                    c1 = (t + 1) * (HG + 1)
                    out_done = t + 1
                    nc.scalar.dma_start(out=out_d.ap()[:, c0:c1],
                                        in_=out_stage[:, c0:c1])

    nc.compile()
    return nc


def _prepare(query, key_cache, value_cache, block_table, seq_lens):
    """Shard FULL inputs into per-core SPMD input maps. Returns
    (in_maps, assign, n16, n8) where assign[c] = [(slot, seq), ...]."""
    import ml_dtypes
    bf16 = ml_dtypes.bfloat16
    fp8 = ml_dtypes.float8_e3m4
    S = query.shape[0]
    lens = [int(x) for x in seq_lens]

    # ---- host-side shard: per-dtype global tile lists (seq, offset, n)
    tiles16, tiles8 = [], []
    for s in range(S):
        L = lens[s]
        dst = tiles8 if L >= FP8_MIN_L else tiles16
        for t0 in range(0, L, TILE_L):
            dst.append((s, t0, min(TILE_L, L - t0)))
    n16 = (len(tiles16) + N_CORES - 1) // N_CORES
    n8 = (len(tiles8) + N_CORES - 1) // N_CORES
    nt = n16 + n8

    # q^T, kv_head-major, pre-scaled: [d, s*32 + h*4 + g]
    q_hg = query.reshape(S, HG, HEAD_SIZE) * np.float32(SCALE)  # [s, hg, d]
    qT_all = np.ascontiguousarray(
        q_hg.reshape(S * HG, HEAD_SIZE).T).astype(bf16)

    # Gather each sequence's valid KV via block_table (the paged layout),
    # transpose K to [d, h, l].
    kseq, vseq = {}, {}
    for s in range(S):
        L = lens[s]
        nblk = (L + BLOCK_SIZE - 1) // BLOCK_SIZE
        blocks = block_table[s, :nblk].astype(np.int64)
        k = key_cache[blocks].reshape(nblk * BLOCK_SIZE, NUM_KV_HEADS,
                                      HEAD_SIZE)[:L]
        v = value_cache[blocks].reshape(nblk * BLOCK_SIZE, NUM_KV_HEADS,
                                        HEAD_SIZE)[:L]
        dt = fp8 if L >= FP8_MIN_L else bf16
        kseq[s] = np.ascontiguousarray(k.transpose(2, 1, 0)).astype(dt)
        vseq[s] = v.reshape(L, NUM_KV_HEADS * HEAD_SIZE).astype(dt)

    in_maps = []
    assign = []  # per core: list of (slot, seq)
    for c in range(N_CORES):
        kv16 = np.zeros((max(n16, 1), 128, KV_COLS), dtype=bf16)
        kv8 = np.zeros((max(n8, 1), 128, KV_COLS), dtype=fp8)
        qc = np.zeros((128, nt * HG), dtype=bf16)
        slots = []

        def fill(kv_all, tiles, cnt, slot0):
            for i in range(cnt):
                gi = c * cnt + i
                if gi >= len(tiles):
                    continue
                s, t0, n = tiles[gi]
                kv = kv_all[i]
                kv[:, :HB].reshape(128, NUM_KV_HEADS, HEAD_SIZE)[
                    :, :, :n] = kseq[s][:, :, t0:t0 + n]
                kv[:n, HB:2 * HB] = vseq[s][t0:t0 + n]
                kv[:n, KV_COLS - 2] = kv.dtype.type(1.0)
                slot = slot0 + i
                qb = slot * HG
                qc[:, qb:qb + HG] = qT_all[:, s * HG:(s + 1) * HG]
                slots.append((slot, s))

        fill(kv16, tiles16, n16, 0)
        fill(kv8, tiles8, n8, n16)
        in_maps.append({
            "kv16": np.ascontiguousarray(
                kv16.transpose(1, 0, 2).reshape(128, -1)),
            "kv8": np.ascontiguousarray(
                kv8.transpose(1, 0, 2).reshape(128, -1)),
            "q": qc,
        })
        assign.append(slots)
    return in_maps, assign, n16, n8


def _combine(results, assign, S, nt):
    """Sum per-tile partial numerators/denominators per sequence, normalize.
    Returns None if the results look corrupted (e.g. a core transiently
    returned zeros -> denominator <= 0), so the caller can retry."""
    num = np.zeros((S, HG, HEAD_SIZE), dtype=np.float64)
    den = np.zeros((S, HG), dtype=np.float64)
    for c in range(N_CORES):
        o = results[c]["out"]  # [128, nt*33]
        for slot, s in assign[c]:
            blk = o[:, slot * (HG + 1):(slot + 1) * (HG + 1)]
            # (rows 32-127 of the denominator column are device scratch)
            if not (np.isfinite(blk[:, :HG]).all()
                    and np.isfinite(blk[:HG, HG]).all()):
                return None
            num[s] += blk[:, :HG].T
            den[s] += blk[:HG, HG]
    if not (den > 0).all():
        return None
    out = (num / den[:, :, None]).astype(np.float32)
    if not np.isfinite(out).all():
        return None
    return out.reshape(S, NUM_HEADS * HEAD_SIZE)


def kernel(query, key_cache, value_cache, block_table, seq_lens):
    query = np.ascontiguousarray(np.asarray(query, dtype=np.float32))
    key_cache = np.asarray(key_cache, dtype=np.float32)
    value_cache = np.asarray(value_cache, dtype=np.float32)
    block_table = np.asarray(block_table, dtype=np.int32)
    seq_lens = np.asarray(seq_lens, dtype=np.int32)

    in_maps, assign, n16, n8 = _prepare(query, key_cache, value_cache,
                                        block_table, seq_lens)

    # bass_utils imports antenv.axon_hooks when tracing is requested; the
    # image's antenv lacks that module, so synthesize a shim defensively.
    try:
        import antenv.axon_hooks  # noqa: F401
    except ImportError:
        try:
            import sys
            import types

            import antenv
            mod = types.ModuleType("antenv.axon_hooks")
            mod._hook = None
            mod.set_axon_ntff_profile_hook = \
                lambda h: setattr(mod, "_hook", h)
            mod.get_axon_ntff_profile_hook = lambda: mod._hook
            sys.modules["antenv.axon_hooks"] = mod
            antenv.axon_hooks = mod
            from trn_agent_boot.trn_boot import _ntff_profile_via_ctypes
            mod._hook = _ntff_profile_via_ctypes("/opt/axon/libaxon_pjrt.so")
        except Exception:  # noqa: BLE001 - tracing is optional
            pass

    from concourse.bass_utils import run_bass_kernel_spmd

    key = (n16, n8)
    if key not in _PROGRAM_CACHE:
        _PROGRAM_CACHE[key] = _build_program(n16, n8)
    nc = _PROGRAM_CACHE[key]

    global LAST_RUN
    out = None
    for attempt in range(3):
        br = run_bass_kernel_spmd(nc, in_maps, list(range(N_CORES)))
        LAST_RUN = br
        out = _combine(br.results, assign, query.shape[0], n16 + n8)
        if out is not None:
            break
        # transient device glitch (a core returned zeros/NaNs) -> retry
    assert out is not None, "device returned corrupted results 3x"
    return out


# revision 10
# speedup vs baseline: 1.0639x; 1.0639x over previous
"""Paged-attention decode (GQA) on 8 Trainium2 NeuronCores.

Strategy (data-parallel over 128-token tiles):
  - Host gathers each sequence's valid KV blocks (via block_table/seq_lens)
    into packed 128-token tiles: K transposed to [D=128, L] per KV head,
    V natural [L, D=128] per KV head, plus a validity column (for the
    softmax denominator matmul).
  - Tiles are distributed evenly across the 8 cores (each tile = same cost).
  - Precision: the kernel is HBM-bandwidth bound, so KV bytes are
    everything. Sequences with L >= 512 tokens ship K/V in fp8 (e3m4:
    4 mantissa bits); shorter sequences (whose softmax averages over
    fewer tokens and so amplifies quantization noise the most) stay in
    bf16. q and p (the exp'd scores) stay bf16 -- the tensor engine
    accepts mixed-dtype operands. Accumulation is fp32 PSUM; the final
    combine runs on host in float64. End-to-end rel err ~1.1e-2
    (gate 2e-2) -- validated offline against the fp64 reference; the
    bf16-only variant of this pipeline reproduced its offline sim
    error to 4 digits on hardware.
  - No masking is needed: padded tokens have K=V=0 so scores=0, p=1,
    but V=0 keeps them out of the numerator and the valid column keeps
    them out of the denominator.
  - Device, per tile: 8 QK matmuls (K_h stationary, q streams) ->
    scores [128L, 32hg] in PSUM, one ScalarE exp -> p bf16, 8 PV
    matmuls (V_h stationary, p streams) + 1 denominator matmul into
    acc [128, 33] PSUM, DVE copy to an SBUF staging buffer. KV streams
    in ~1-2 MiB DMA chunks; finished outputs stream back incrementally.
  - Host sums per-tile partial numerators/denominators per sequence and
    normalizes (the standard distributed-softmax combine).
"""

import math

import numpy as np

# Problem constants (hardcoded per task contract).
NUM_SEQS = 32
NUM_HEADS = 32
NUM_KV_HEADS = 8
GQA = NUM_HEADS // NUM_KV_HEADS  # 4
HEAD_SIZE = 128
BLOCK_SIZE = 16
MAX_BLOCKS_PER_SEQ = 128
MAX_SEQ_LEN = MAX_BLOCKS_PER_SEQ * BLOCK_SIZE
SCALE = 1.0 / math.sqrt(HEAD_SIZE)
N_CORES = 8
TILE_L = 128          # tokens per device tile
FP8_MIN_L = 512       # sequences at least this long ship KV in fp8
HG = NUM_HEADS        # 32 (kv_head-major query head order)
HB = NUM_KV_HEADS * HEAD_SIZE      # 1024 cols per K/V plane
KV_COLS = 2 * HB + 2               # 2050: K | V | valid | pad

_PROGRAM_CACHE = {}
LAST_RUN = None  # BassKernelResults of the most recent run (for test harness)


def _build_program(n16: int, n8: int):
    """Build the SPMD Bass/Tile program: per core, n16 bf16 KV tiles
    followed by n8 fp8(e3m4) KV tiles."""
    import concourse.bacc as bacc
    import concourse.mybir as mybir
    import concourse.tile as tile

    f32 = mybir.dt.float32
    bf16 = mybir.dt.bfloat16
    fp8 = mybir.dt.float8e3
    nt = n16 + n8
    nc = bacc.Bacc("TRN2", target_bir_lowering=False, debug=False,
                   num_devices=N_CORES)

    kv16_d = nc.dram_tensor("kv16", [128, max(n16, 1) * KV_COLS], bf16,
                            kind="ExternalInput")
    kv8_d = nc.dram_tensor("kv8", [128, max(n8, 1) * KV_COLS], fp8,
                           kind="ExternalInput")
    q_d = nc.dram_tensor("q", [128, nt * HG], bf16, kind="ExternalInput")
    out_d = nc.dram_tensor("out", [128, nt * (HG + 1)], f32,
                           kind="ExternalOutput")

    with tile.TileContext(nc) as tc:
        with (
            tc.tile_pool(name="const", bufs=1) as const_pool,
            tc.tile_pool(name="kv16p", bufs=1) as kv16_pool,
            tc.tile_pool(name="kv8p", bufs=16) as kv8_pool,
            tc.tile_pool(name="pp", bufs=4) as p_pool,
            tc.tile_pool(name="acc_sb", bufs=1) as stage_pool,
            tc.tile_pool(name="ps_s", bufs=4, space="PSUM") as ps_scores,
            tc.tile_pool(name="ps_o", bufs=4, space="PSUM") as ps_acc,
        ):
            # q first on the same (sync/HWDGE) queue as KV so it does not
            # compete with the KV stream for HBM bandwidth mid-kernel.
            qt = const_pool.tile([128, nt * HG], bf16)
            nc.sync.dma_start(out=qt[:], in_=q_d.ap())
            out_stage = stage_pool.tile([128, nt * (HG + 1)], f32)
            nc.vector.memset(out_stage[:], 0.0)

            # DMA chunk schedule. bf16 tiles (if any) come first in one
            # chunk; fp8 tiles stream in 4-tile (~1 MiB) chunks, tapering
            # to 2/1-tile chunks at the end. The whole per-core KV stream
            # (~80 KB/partition) fits in SBUF, so every chunk gets its own
            # buffer (pool bufs >= chunk count): the DMA engine never
            # waits on buffer reuse and runs ahead at full HBM rate while
            # TensorE drains tiles from SBUF at its own pace.
            sizes = []
            r = n8
            while r > 5:
                sizes.append(4)
                r -= 4
            sizes += {5: [2, 2, 1], 4: [2, 1, 1], 3: [2, 1],
                      2: [1, 1], 1: [1], 0: []}[r]
            starts = [sum(sizes[:i]) for i in range(len(sizes))]

            chunk_tiles = {}
            if n16:
                ct = kv16_pool.tile([128, n16 * KV_COLS], bf16)
                nc.sync.dma_start(out=ct[:], in_=kv16_d.ap())
                for i in range(n16):
                    chunk_tiles[i] = ct[:, i * KV_COLS:(i + 1) * KV_COLS]
            big = max(sizes) if sizes else 1
            for ci, (sz, st) in enumerate(zip(sizes, starts)):
                ct = kv8_pool.tile([128, big * KV_COLS], fp8)
                c0 = st * KV_COLS
                if ci >= len(sizes) - 2 and sz == 1:
                    # split the last tiles' DMA into K-plane then V-plane
                    # so their QK matmuls overlap the V transfer
                    # (shortens the end-of-kernel serial drain)
                    nc.sync.dma_start(
                        out=ct[:, :HB],
                        in_=kv8_d.ap()[:, c0:c0 + HB])
                    nc.sync.dma_start(
                        out=ct[:, HB:KV_COLS],
                        in_=kv8_d.ap()[:, c0 + HB:c0 + KV_COLS])
                else:
                    nc.sync.dma_start(
                        out=ct[:, :sz * KV_COLS],
                        in_=kv8_d.ap()[:, c0:c0 + sz * KV_COLS])
                for i in range(sz):
                    chunk_tiles[n16 + st + i] = ct[:, i * KV_COLS:
                                                   (i + 1) * KV_COLS]

            OUT_CHUNK = 8  # tiles per incremental output store
            out_done = 0   # tiles whose output has been stored

            for t in range(nt):
                kvt = chunk_tiles[t]

                # scores[l, h*4+g] = sum_d K[l,d] * q_scaled[h,g,d]
                scores = ps_scores.tile([128, HG], f32)
                qb = t * HG
                for h in range(NUM_KV_HEADS):
                    nc.tensor.matmul(
                        scores[:, h * GQA:(h + 1) * GQA],
                        kvt[:, h * HEAD_SIZE:(h + 1) * HEAD_SIZE],
                        qt[:, qb + h * GQA:qb + (h + 1) * GQA],
                        start=True, stop=True)

                # p = exp(scores), emitted directly in bf16
                p = p_pool.tile([128, HG], bf16)
                nc.scalar.activation(
                    p[:], scores[:], mybir.ActivationFunctionType.Exp)

                # acc[d, h*4+g] = sum_l V[l, h, d] * p[l, h*4+g]
                # acc[0:32, 32] = per-(h,g) denominator sum_l p[l,:]*valid[l]
                acc = ps_acc.tile([128, HG + 1], f32)
                for h in range(NUM_KV_HEADS):
                    nc.tensor.matmul(
                        acc[:, h * GQA:(h + 1) * GQA],
                        kvt[:, HB + h * HEAD_SIZE:HB + (h + 1) * HEAD_SIZE],
                        p[:, h * GQA:(h + 1) * GQA],
                        start=True, stop=True)
                valid = kvt[:, KV_COLS - 2:KV_COLS - 1]
                nc.tensor.matmul(acc[0:HG, HG:HG + 1], p[:], valid,
                                 start=True, stop=True)

                base = t * (HG + 1)
                # single copy; rows 32-127 of the denominator column are
                # PSUM garbage, skipped by the host combine
                nc.vector.tensor_copy(
                    out_stage[:, base:base + HG + 1], acc[:])

                # stream finished output chunks while KV is still loading
                # (stores stay >= 4 tiles so per-partition DMA segments
                # stay above the 512B descriptor efficiency floor)
                emit = (t % OUT_CHUNK == OUT_CHUNK - 1 or t == nt - 1)
                if emit:
                    c0 = out_done * (HG + 1)
                    c1 = (t + 1) * (HG + 1)
                    out_done = t + 1
                    nc.scalar.dma_start(out=out_d.ap()[:, c0:c1],
                                        in_=out_stage[:, c0:c1])

    nc.compile()
    return nc


def _prepare(query, key_cache, value_cache, block_table, seq_lens):
    """Shard FULL inputs into per-core SPMD input maps. Returns
    (in_maps, assign, n16, n8) where assign[c] = [(slot, seq), ...]."""
    import ml_dtypes
    bf16 = ml_dtypes.bfloat16
    fp8 = ml_dtypes.float8_e3m4
    S = query.shape[0]
    lens = [int(x) for x in seq_lens]

    # ---- host-side shard: per-dtype global tile lists (seq, offset, n)
    tiles16, tiles8 = [], []
    for s in range(S):
        L = lens[s]
        dst = tiles8 if L >= FP8_MIN_L else tiles16
        for t0 in range(0, L, TILE_L):
            dst.append((s, t0, min(TILE_L, L - t0)))
    n16 = (len(tiles16) + N_CORES - 1) // N_CORES
    n8 = (len(tiles8) + N_CORES - 1) // N_CORES
    nt = n16 + n8

    # q^T, kv_head-major, pre-scaled: [d, s*32 + h*4 + g]
    q_hg = query.reshape(S, HG, HEAD_SIZE) * np.float32(SCALE)  # [s, hg, d]
    qT_all = np.ascontiguousarray(
        q_hg.reshape(S * HG, HEAD_SIZE).T).astype(bf16)

    # Gather each sequence's valid KV via block_table (the paged layout),
    # transpose K to [d, h, l].
    kseq, vseq = {}, {}
    for s in range(S):
        L = lens[s]
        nblk = (L + BLOCK_SIZE - 1) // BLOCK_SIZE
        blocks = block_table[s, :nblk].astype(np.int64)
        k = key_cache[blocks].reshape(nblk * BLOCK_SIZE, NUM_KV_HEADS,
                                      HEAD_SIZE)[:L]
        v = value_cache[blocks].reshape(nblk * BLOCK_SIZE, NUM_KV_HEADS,
                                        HEAD_SIZE)[:L]
        dt = fp8 if L >= FP8_MIN_L else bf16
        kseq[s] = np.ascontiguousarray(k.transpose(2, 1, 0)).astype(dt)
        vseq[s] = v.reshape(L, NUM_KV_HEADS * HEAD_SIZE).astype(dt)

    in_maps = []
    assign = []  # per core: list of (slot, seq)
    for c in range(N_CORES):
        kv16 = np.zeros((max(n16, 1), 128, KV_COLS), dtype=bf16)
        kv8 = np.zeros((max(n8, 1), 128, KV_COLS), dtype=fp8)
        qc = np.zeros((128, nt * HG), dtype=bf16)
        slots = []

        def fill(kv_all, tiles, cnt, slot0):
            for i in range(cnt):
                gi = c * cnt + i
                if gi >= len(tiles):
                    continue
                s, t0, n = tiles[gi]
                kv = kv_all[i]
                kv[:, :HB].reshape(128, NUM_KV_HEADS, HEAD_SIZE)[
                    :, :, :n] = kseq[s][:, :, t0:t0 + n]
                kv[:n, HB:2 * HB] = vseq[s][t0:t0 + n]
                kv[:n, KV_COLS - 2] = kv.dtype.type(1.0)
                slot = slot0 + i
                qb = slot * HG
                qc[:, qb:qb + HG] = qT_all[:, s * HG:(s + 1) * HG]
                slots.append((slot, s))

        fill(kv16, tiles16, n16, 0)
        fill(kv8, tiles8, n8, n16)
        in_maps.append({
            "kv16": np.ascontiguousarray(
                kv16.transpose(1, 0, 2).reshape(128, -1)),
            "kv8": np.ascontiguousarray(
                kv8.transpose(1, 0, 2).reshape(128, -1)),
            "q": qc,
        })
        assign.append(slots)
    return in_maps, assign, n16, n8


def _combine(results, assign, S, nt):
    """Sum per-tile partial numerators/denominators per sequence, normalize.
    Returns None if the results look corrupted (e.g. a core transiently
    returned zeros -> denominator <= 0), so the caller can retry."""
    num = np.zeros((S, HG, HEAD_SIZE), dtype=np.float64)
    den = np.zeros((S, HG), dtype=np.float64)
    for c in range(N_CORES):
        o = results[c]["out"]  # [128, nt*33]
        for slot, s in assign[c]:
            blk = o[:, slot * (HG + 1):(slot + 1) * (HG + 1)]
            # (rows 32-127 of the denominator column are device scratch)
            if not (np.isfinite(blk[:, :HG]).all()
                    and np.isfinite(blk[:HG, HG]).all()):
                return None
            num[s] += blk[:, :HG].T
            den[s] += blk[:HG, HG]
    if not (den > 0).all():
        return None
    out = (num / den[:, :, None]).astype(np.float32)
    if not np.isfinite(out).all():
        return None
    return out.reshape(S, NUM_HEADS * HEAD_SIZE)


def kernel(query, key_cache, value_cache, block_table, seq_lens):
    query = np.ascontiguousarray(np.asarray(query, dtype=np.float32))
    key_cache = np.asarray(key_cache, dtype=np.float32)
    value_cache = np.asarray(value_cache, dtype=np.float32)
    block_table = np.asarray(block_table, dtype=np.int32)
    seq_lens = np.asarray(seq_lens, dtype=np.int32)

    in_maps, assign, n16, n8 = _prepare(query, key_cache, value_cache,
                                        block_table, seq_lens)

    # bass_utils imports antenv.axon_hooks when tracing is requested; the
    # image's antenv lacks that module, so synthesize a shim defensively.
    try:
        import antenv.axon_hooks  # noqa: F401
    except ImportError:
        try:
            import sys
            import types

            import antenv
            mod = types.ModuleType("antenv.axon_hooks")
            mod._hook = None
            mod.set_axon_ntff_profile_hook = \
                lambda h: setattr(mod, "_hook", h)
            mod.get_axon_ntff_profile_hook = lambda: mod._hook
            sys.modules["antenv.axon_hooks"] = mod
            antenv.axon_hooks = mod
            from trn_agent_boot.trn_boot import _ntff_profile_via_ctypes
            mod._hook = _ntff_profile_via_ctypes("/opt/axon/libaxon_pjrt.so")
        except Exception:  # noqa: BLE001 - tracing is optional
            pass

    from concourse.bass_utils import run_bass_kernel_spmd

    key = (n16, n8)
    if key not in _PROGRAM_CACHE:
        _PROGRAM_CACHE[key] = _build_program(n16, n8)
    nc = _PROGRAM_CACHE[key]

    global LAST_RUN
    out = None
    for attempt in range(3):
        br = run_bass_kernel_spmd(nc, in_maps, list(range(N_CORES)))
        LAST_RUN = br
        out = _combine(br.results, assign, query.shape[0], n16 + n8)
        if out is not None:
            break
        # transient device glitch (a core returned zeros/NaNs) -> retry
    assert out is not None, "device returned corrupted results 3x"
    return out
